# revision 1
# baseline (speedup 1.0000x reference)
"""Trainium2 Bass kernel: attention with additive bias + masked_fill(1e-4).

Sharding: pure data-parallel, one batch element per NeuronCore (B=8, 8 cores).

Math (per batch element b, per head h):
  s[q,k]   = (h@Wq*SCALE)[q]·(h@Wk)[k] + bias[q,k,h]
  p_true   = where(mask[q,k], exp(1e-4-ish const), exp(s))   (softmax numerator)
  out      = (p_true @ V / rowsum(p_true)) @ Wo

Device trick: host folds -30000*mask into the (pre-transposed) bias so
exp(s) == 0 exactly at masked positions; the masked constant contribution
e_c = exp(1e-4) is restored by accumulating V_aug^T @ (e_c*mask^T) into the
same PSUM group as V_aug^T @ exp(s^T).  V_aug has a ones column per head so
row 64 of each output group is the softmax denominator.

All matmuls run as float32r (full-rate fp32 mode, needs moving dim >= 256).
"""

import sys

sys.path.insert(0, "/opt/trn_rl_repo")

from contextlib import ExitStack

import numpy as np

import concourse.bass as bass
import concourse.bacc as bacc
import concourse.tile as tile
from concourse import mybir
from concourse.bass_utils import run_bass_kernel_spmd

F32 = mybir.dt.float32
F32R = mybir.dt.float32r
AF = mybir.ActivationFunctionType

S, D, H, DH = 1024, 768, 12, 64
P = 128
ND = D // P          # 6 chunks of 128 along D (and along hd)
NK = S // P          # 8 chunks of 128 along k / s
NQ = 2               # q chunks of 512
QW = S // NQ         # 512
HW = 384             # half of hd for N<=512 matmuls
SCALE = DH ** -0.5
BIG = 30000.0
EC = float(np.exp(np.float32(1e-4)))


def mmr(nc, out, lhsT, rhs, **kw):
    nc.tensor.matmul(out, lhsT, rhs, **kw)


def build():
    nc = bacc.Bacc("TRN2", target_bir_lowering=False)
    hT = nc.dram_tensor("hT", [D, S], F32R, kind="ExternalInput")
    biasT = nc.dram_tensor("biasT", [H, S, S], F32R, kind="ExternalInput")
    maskT = nc.dram_tensor("maskT", [S, S], F32R, kind="ExternalInput")
    wq = nc.dram_tensor("wq", [D, D], F32R, kind="ExternalInput")
    wk = nc.dram_tensor("wk", [D, D], F32R, kind="ExternalInput")
    wv = nc.dram_tensor("wv", [D, D], F32R, kind="ExternalInput")
    wo = nc.dram_tensor("wo", [D, D], F32R, kind="ExternalInput")
    identD = nc.dram_tensor("ident", [P, P], F32R, kind="ExternalInput")
    onesD = nc.dram_tensor("ones64", [1, 64], F32R, kind="ExternalInput")
    onescolD = nc.dram_tensor("onescols", [P, H], F32R, kind="ExternalInput")
    out = nc.dram_tensor("out", [S, D], F32, kind="ExternalOutput")

    with tile.TileContext(nc) as tc, ExitStack() as ctx:
        wp = ctx.enter_context(tc.tile_pool(name="wp", bufs=1))
        hp = ctx.enter_context(tc.tile_pool(name="hp", bufs=1))
        ktp = ctx.enter_context(tc.tile_pool(name="ktp", bufs=1))
        qtp = ctx.enter_context(tc.tile_pool(name="qtp", bufs=1))
        vp = ctx.enter_context(tc.tile_pool(name="vp", bufs=1))
        mkp = ctx.enter_context(tc.tile_pool(name="mkp", bufs=1))
        atp = ctx.enter_context(tc.tile_pool(name="atp", bufs=1))
        cst = ctx.enter_context(tc.tile_pool(name="cst", bufs=1))
        bsp = ctx.enter_context(tc.tile_pool(name="bsp", bufs=6))
        pzp = ctx.enter_context(tc.tile_pool(name="pzp", bufs=8))
        obp = ctx.enter_context(tc.tile_pool(name="obp", bufs=2))
        nrm = ctx.enter_context(tc.tile_pool(name="nrm", bufs=2))
        ps_s = ctx.enter_context(tc.tile_pool(name="ps_s", bufs=6, space="PSUM"))
        ps_o = ctx.enter_context(tc.tile_pool(name="ps_o", bufs=2, space="PSUM"))

        ident = cst.tile([P, P], F32R, name="ident", tag="ident")
        nc.sync.dma_start(ident[:], identD[:, :])
        ones64 = cst.tile([1, 64], F32R, name="ones64", tag="ones64")
        nc.sync.dma_start(ones64[:], onesD[:, :])

        # warm-up matmuls: absorb first-use semaphore waits for each PSUM pool
        # (walrus limits sync-wait commands per LDWEIGHTS)
        wu1 = ps_s.tile([P, P], F32, name="wu1", tag="s")
        mmr(nc, wu1[:], ident[:], ident[:], start=True, stop=True)
        wu2 = ps_o.tile([65, P], F32, name="wu2", tag="o")
        mmr(nc, wu2[:], ident[:, 0:65], ident[:], start=True, stop=True)

        # ---- load weights and hT -------------------------------------------------
        wq_t, wk_t, wv_t = [], [], []
        for nm, dram, lst in (("wq", wq, wq_t), ("wk", wk, wk_t), ("wv", wv, wv_t)):
            for i in range(ND):
                t = wp.tile([P, D], F32R, name=f"{nm}{i}", tag=f"{nm}{i}")
                nc.sync.dma_start(t[:], dram[i * P:(i + 1) * P, :])
                lst.append(t)
        hT_t = []
        for i in range(ND):
            t = hp.tile([P, S], F32R, name=f"h{i}", tag=f"h{i}")
            nc.sync.dma_start(t[:], hT[i * P:(i + 1) * P, :])
            hT_t.append(t)

        # ---- A: K^T [d, k] full --------------------------------------------------
        kT_t = [ktp.tile([P, S], F32R, name=f"kt{i}", tag=f"kt{i}") for i in range(ND)]
        for i in range(ND):
            for sc in range(NQ):
                ps = ps_s.tile([P, QW], F32, name="s", tag="s")
                for Dc in range(ND):
                    mmr(nc, ps[:], wk_t[Dc][:, i * P:(i + 1) * P],
                        hT_t[Dc][:, sc * QW:(sc + 1) * QW],
                        start=(Dc == 0), stop=(Dc == ND - 1))
                nc.vector.tensor_copy(kT_t[i][:, sc * QW:(sc + 1) * QW], ps[:])

        # ---- A: V_aug [s, 65*H] (per head: 64 V cols then a ones col) ------------
        va_t = []
        for sc in range(NK):
            t = vp.tile([P, 65 * H], F32R, name=f"va{sc}", tag=f"va{sc}")
            ones_cols = t.rearrange("p (h c) -> p h c", c=65)[:, :, 64]
            nc.sync.dma_start(ones_cols, onescolD[:, :])
            va_t.append(t)
        for sc in range(NK):
            for half in range(2):
                ps = ps_s.tile([P, HW], F32, name="s", tag="s")
                for Dc in range(ND):
                    mmr(nc, ps[:], hT_t[Dc][:, sc * P:(sc + 1) * P],
                        wv_t[Dc][:, half * HW:(half + 1) * HW],
                        start=(Dc == 0), stop=(Dc == ND - 1))
                for j in range(6):
                    hh = half * 6 + j
                    nc.vector.tensor_copy(
                        va_t[sc][:, 65 * hh:65 * hh + 64],
                        ps[:, j * 64:(j + 1) * 64])

        # ---- wo: load once, reusing the wv slots (wv is dead after phase A) ------
        wo_t = []
        for i in range(ND):
            t = wp.tile([P, D], F32R, name=f"wo{i}", tag=f"wv{i}")
            nc.sync.dma_start(t[:], wo[i * P:(i + 1) * P, :])
            wo_t.append(t)

        # ---- main loop over q chunks --------------------------------------------
        for qc in range(NQ):
            q0 = qc * QW
            # Q^T [d, q-chunk]
            qT_t = [qtp.tile([P, QW], F32R, name=f"qt{i}", tag=f"qt{i}") for i in range(ND)]
            for i in range(ND):
                ps = ps_s.tile([P, QW], F32, name="s", tag="s")
                for Dc in range(ND):
                    mmr(nc, ps[:], wq_t[Dc][:, i * P:(i + 1) * P],
                        hT_t[Dc][:, q0:q0 + QW],
                        start=(Dc == 0), stop=(Dc == ND - 1))
                nc.vector.tensor_copy(qT_t[i][:], ps[:])
            # mask^T (pre-scaled by e_c) for this q chunk
            mk_t = []
            for k in range(NK):
                t = mkp.tile([P, QW], F32R, name=f"mk{k}", tag=f"mk{k}")
                nc.sync.dma_start(t[:], maskT[k * P:(k + 1) * P, q0:q0 + QW])
                mk_t.append(t)

            at_t = [atp.tile([P, QW], F32R, name=f"at{i}", tag=f"at{i}") for i in range(ND)]

            for h in range(H):
                ti, ro = h // 2, (h % 2) * 64
                o_ps = ps_o.tile([65, QW], F32, name="o", tag="o")
                for k in range(NK):
                    bt = bsp.tile([P, QW], F32R, name="bias", tag="bias")
                    nc.sync.dma_start(
                        bt[:], biasT[h, k * P:(k + 1) * P, q0:q0 + QW])
                    s_ps = ps_s.tile([P, QW], F32, name="s", tag="s")
                    mmr(nc, s_ps[:],
                        kT_t[ti][ro:ro + 64, k * P:(k + 1) * P],
                        qT_t[ti][ro:ro + 64, :],
                        start=True, stop=False)
                    mmr(nc, s_ps[:], ident[:], bt[:], start=False, stop=True)
                    pz = pzp.tile([P, QW], F32R, name="pz", tag="pz")
                    nc.scalar.activation(pz[:], s_ps[:], AF.Exp)
                    mmr(nc, o_ps[:], va_t[k][:, 65 * h:65 * h + 65], pz[:],
                        start=(k == 0), stop=False, skip_group_check=True)
                    mmr(nc, o_ps[:], va_t[k][:, 65 * h:65 * h + 65], mk_t[k][:],
                        start=False, stop=(k == NK - 1), skip_group_check=True)
                # normalize: rows 0:64 are numerator^T, row 64 is denominator
                rc = nrm.tile([1, QW], F32R, name="rc", tag="rc")
                with nc.allow_low_precision(reason="f32r is fp32-width"):
                    nc.vector.reciprocal(rc[:], o_ps[64:65, :])
                bc_ps = ps_s.tile([64, QW], F32, name="s", tag="s")
                mmr(nc, bc_ps[:], ones64[:], rc[:], start=True, stop=True)
                bc = nrm.tile([64, QW], F32, name="bc", tag="bc")
                nc.scalar.copy(bc[:], bc_ps[:])
                nc.vector.tensor_mul(at_t[ti][ro:ro + 64, :], o_ps[0:64, :], bc[:])

            # ---- out projection for this q chunk ---------------------------------
            for qs in range(QW // P):
                for half in range(2):
                    ps = ps_s.tile([P, HW], F32, name="s", tag="s")
                    for i in range(ND):
                        mmr(nc, ps[:], at_t[i][:, qs * P:(qs + 1) * P],
                            wo_t[i][:, half * HW:(half + 1) * HW],
                            start=(i == 0), stop=(i == ND - 1))
                    ot = obp.tile([P, HW], F32, name="ob", tag="ob")
                    nc.vector.tensor_copy(ot[:], ps[:])
                    nc.sync.dma_start(
                        out[q0 + qs * P:q0 + (qs + 1) * P,
                            half * HW:(half + 1) * HW], ot[:])
    nc.finalize()
    return nc


_NC = None


def kernel(h, att_bias, mask, Wq, Wk, Wv, Wo):
    global _NC
    h = np.asarray(h, dtype=np.float32)
    att_bias = np.asarray(att_bias, dtype=np.float32)
    mask_f = np.asarray(mask).astype(np.float32)
    B = h.shape[0]

    hT = np.ascontiguousarray(h.transpose(0, 2, 1))                 # [B, D, S]
    biasT = np.ascontiguousarray(att_bias.transpose(0, 3, 2, 1))    # [B, H, k, q]
    mT = np.ascontiguousarray(mask_f.transpose(0, 2, 1))            # [B, k, q]
    biasT -= BIG * mT[:, None, :, :]
    mT_ec = mT * EC
    wq_s = np.ascontiguousarray((np.asarray(Wq, np.float32) * SCALE))
    wk_ = np.ascontiguousarray(np.asarray(Wk, np.float32))
    wv_ = np.ascontiguousarray(np.asarray(Wv, np.float32))
    wo_ = np.ascontiguousarray(np.asarray(Wo, np.float32))

    if _NC is None:
        _NC = build()
    in_maps = [
        {"hT": hT[b], "biasT": biasT[b], "maskT": mT_ec[b],
         "wq": wq_s, "wk": wk_, "wv": wv_, "wo": wo_,
         "ident": np.eye(128, dtype=np.float32),
         "ones64": np.ones((1, 64), dtype=np.float32),
         "onescols": np.ones((128, 12), dtype=np.float32)}
        for b in range(B)
    ]
    res = run_bass_kernel_spmd(_NC, in_maps, core_ids=list(range(B)))
    return np.stack([r["out"] for r in res.results], axis=0)


if __name__ == "__main__":
    rng = np.random.default_rng(0)
    inputs = {
        "h": rng.standard_normal((8, S, D), dtype=np.float32),
        "att_bias": rng.standard_normal((8, S, S, H), dtype=np.float32),
        "mask": rng.integers(0, 2, (8, S, S)).astype(bool),
        "Wq": rng.standard_normal((D, D), dtype=np.float32) * D ** -0.5,
        "Wk": rng.standard_normal((D, D), dtype=np.float32) * D ** -0.5,
        "Wv": rng.standard_normal((D, D), dtype=np.float32) * D ** -0.5,
        "Wo": rng.standard_normal((D, D), dtype=np.float32) * D ** -0.5,
    }
    print(kernel(**inputs).shape)

